# revision 1
# baseline (speedup 1.0000x reference)
"""GATv2 x5 (gnn_message_passing) on 8 Trainium2 NeuronCores.

Sharding: nodes partitioned across 8 cores by destination-node owner
(6250 nodes/core, padded to 6272 = 49 tiles of 128). Edges live with
their dst owner, sorted by dst, packed into 128-edge chunks per
dst-tile. Per layer: each core computes xl/xr for its own nodes,
AllGathers xl (the only cross-core exchange), then does
gather -> GATv2 score -> segment softmax -> scatter-add locally via
PE matmuls with runtime-built one-hot matrices.
"""
import sys
import numpy as np

sys.path.insert(0, "/opt/trn_rl_repo")

import concourse.bass as bass
import concourse.bacc as bacc
import concourse.mybir as mybir
import concourse.tile as tile
from concourse.bass_utils import run_bass_kernel_spmd
from concourse.masks import make_identity

F32 = mybir.dt.float32
I32 = mybir.dt.int32
AF = mybir.ActivationFunctionType
OP = mybir.AluOpType

N = 50000
DIN = 7
D = 128
T = 5
CORES = 8
SH = N // CORES            # 6250 nodes per core
TILES = 49
SHP = TILES * 128          # 6272 padded nodes per core
NPG = CORES * SHP          # 50176 global padded node space
NEG = 0.2


def _build_nc(K: int):
    nc = bacc.Bacc("TRN2", target_bir_lowering=False, debug=False,
                   num_devices=CORES)

    xT_full = nc.dram_tensor("xT_full", [DIN, NPG], F32, kind="ExternalInput")
    xT_own = nc.dram_tensor("xT_own", [DIN, SHP], F32, kind="ExternalInput")
    Wlr0 = nc.dram_tensor("Wlr0", [DIN, 2 * D], F32, kind="ExternalInput")
    Wlr = nc.dram_tensor("Wlr", [T - 1, D, 2 * D], F32, kind="ExternalInput")
    brow2 = nc.dram_tensor("brow2", [T, D], F32, kind="ExternalInput")
    bout = nc.dram_tensor("bout", [D, T], F32, kind="ExternalInput")
    attw = nc.dram_tensor("attw", [T, D], F32, kind="ExternalInput")
    src_i = nc.dram_tensor("src_i", [128, TILES * K], I32, kind="ExternalInput")
    dstr_i = nc.dram_tensor("dstr_i", [128, TILES * K], I32, kind="ExternalInput")
    dstl_f = nc.dram_tensor("dstl_f", [128, TILES * K], F32, kind="ExternalInput")
    dstl_r = nc.dram_tensor("dstl_r", [1, TILES * K * 128], F32, kind="ExternalInput")

    out_t = nc.dram_tensor("out", [SHP, D], F32, kind="ExternalOutput")

    with tile.TileContext(nc) as tc:
        with (
            tc.tile_pool(name="pers", bufs=1) as pers,
            tc.tile_pool(name="wl", bufs=2) as wl,
            tc.tile_pool(name="edge", bufs=2) as ep,
            tc.tile_pool(name="oh", bufs=4) as ohp,
            tc.tile_pool(name="sb", bufs=3) as sbp,
            tc.tile_pool(name="ps", bufs=2, space="PSUM") as psp,
            tc.tile_pool(name="pst", bufs=2, space="PSUM") as pst,
            tc.tile_pool(name="dram", bufs=2, space="DRAM") as dp,
        ):
            # --- persistent setup ---
            iota_r = pers.tile([128, 128], I32)
            nc.gpsimd.iota(iota_r[:], pattern=[[1, 128]], base=0,
                           channel_multiplier=0)
            iota_c1 = pers.tile([128, 1], I32)
            nc.gpsimd.iota(iota_c1[:], pattern=[[0, 1]], base=0,
                           channel_multiplier=1)
            iota_c1f = pers.tile([128, 1], F32)
            nc.vector.tensor_copy(iota_c1f[:], iota_c1[:])
            xr_all = pers.tile([128, TILES * 128], F32)
            ident = pers.tile([128, 128], F32)
            make_identity(nc, ident[:])
            src_sb = pers.tile([128, TILES * K], I32)
            nc.sync.dma_start(out=src_sb[:], in_=src_i[:, :])
            dstl_sb = pers.tile([128, TILES * K], F32)
            nc.sync.dma_start(out=dstl_sb[:], in_=dstl_f[:, :])
            xT_sb = pers.tile([DIN, SHP], F32)
            nc.sync.dma_start(out=xT_sb[:], in_=xT_own[:, :])
            hT = pers.tile([128, SHP], F32)

            for l in range(T):
                # --- per-layer constants ---
                w_sb = wl.tile([128, 2 * D], F32, tag="w")
                if l == 0:
                    nc.sync.dma_start(out=w_sb[:DIN, :], in_=Wlr0[:, :])
                else:
                    nc.sync.dma_start(out=w_sb[:], in_=Wlr[l - 1, :, :])
                a_b = wl.tile([128, 128], F32, tag="ab")
                nc.sync.dma_start(
                    out=a_b[:], in_=attw[l : l + 1, :].partition_broadcast(128))
                br2_b = wl.tile([128, 128], F32, tag="br2")
                nc.sync.dma_start(
                    out=br2_b[:], in_=brow2[l : l + 1, :].partition_broadcast(128))
                bo_col = wl.tile([128, 1], F32, tag="boc")
                nc.sync.dma_start(out=bo_col[:], in_=bout[:, l : l + 1])
                if l == T - 1:
                    bo_b = wl.tile([128, 128], F32, tag="bob")
                    nc.sync.dma_start(
                        out=bo_b[:],
                        in_=bout[:, l : l + 1].transpose([1, 0]).partition_broadcast(128))

                # --- prologue: xl_own / xr_own, then AllGather xl ---
                xl_cc = dp.tile([SHP, D], F32, tag="xlcc")
                for m in range(TILES):
                    ps2 = psp.tile([128, 2 * D], F32, space="PSUM", tag="pro")
                    if l == 0:
                        lhsT = xT_sb[:, m * 128 : (m + 1) * 128]
                        rhs = w_sb[:DIN, :]
                    else:
                        lhsT = hT[:, m * 128 : (m + 1) * 128]
                        rhs = w_sb[:, :]
                    nc.tensor.matmul(out=ps2[:], lhsT=lhsT, rhs=rhs,
                                     start=True, stop=True)
                    xl_sb = sbp.tile([128, D], F32, tag="xls")
                    nc.scalar.activation(out=xl_sb[:], in_=ps2[:, :D],
                                         func=AF.Identity)
                    nc.sync.dma_start(
                        out=xl_cc[m * 128 : (m + 1) * 128, :], in_=xl_sb[:])
                    nc.vector.tensor_tensor(
                        out=xr_all[:, m * 128 : (m + 1) * 128],
                        in0=ps2[:, D:], in1=br2_b[:], op=OP.add)

                xl_full = dp.tile([NPG, D], F32, tag="xlfull")
                nc.gpsimd.collective_compute(
                    "AllGather",
                    OP.bypass,
                    replica_groups=[list(range(CORES))],
                    ins=[xl_cc[:, :].opt()],
                    outs=[xl_full[:, :].opt()],
                )

                # --- edge stage: per dst-tile ---
                for t in range(TILES):
                    XL = ep.tile([128, K, D + 1], F32, tag="XL")
                    nc.vector.memset(XL[:, :, D : D + 1], 1.0)
                    for k in range(K):
                        col = t * K + k
                        nc.gpsimd.indirect_dma_start(
                            out=XL[:, k, :D], out_offset=None,
                            in_=xl_full[:, :],
                            in_offset=bass.IndirectOffsetOnAxis(
                                ap=src_sb[:, col : col + 1], axis=0),
                        )
                    dstl_b = ep.tile([128, K * 128], F32, tag="dstlb")
                    nc.sync.dma_start(
                        out=dstl_b[:],
                        in_=dstl_r[0:1, t * K * 128 : (t + 1) * K * 128]
                        .partition_broadcast(128))
                    maskT = ep.tile([128, K * 128], F32, tag="maskT")
                    nc.vector.tensor_scalar(
                        out=maskT[:], in0=dstl_b[:], scalar1=iota_c1f[:],
                        scalar2=None, op0=OP.is_equal)
                    S = ep.tile([128, K, D], F32, tag="S")
                    xr_t = xr_all[:, t * 128 : (t + 1) * 128]
                    for g0 in range(0, K, 2):
                        gw = min(2, K - g0)
                        ps_xr = pst.tile([128, 2, D], F32, space="PSUM",
                                         tag="xrp")
                        for j in range(gw):
                            k = g0 + j
                            nc.tensor.matmul(
                                out=ps_xr[:, j, :],
                                lhsT=maskT[:, k * 128 : (k + 1) * 128],
                                rhs=xr_t, start=True, stop=True)
                        nc.vector.tensor_tensor(
                            out=S[:, g0 : g0 + gw, :],
                            in0=XL[:, g0 : g0 + gw, :D],
                            in1=ps_xr[:, :gw, :], op=OP.add)
                    L = ep.tile([128, K, D], F32, tag="L")
                    nc.scalar.activation(
                        out=L[:, :, :], in_=S[:, :, :], func=AF.Prelu, alpha=NEG)
                    e_t = sbp.tile([128, K], F32, tag="e")
                    scr = sbp.tile([128, 128], F32, tag="scr")
                    for k in range(K):
                        nc.vector.scalar_tensor_tensor(
                            out=scr[:], in0=L[:, k, :], scalar=1.0,
                            in1=a_b[:], op0=OP.mult, op1=OP.mult,
                            accum_out=e_t[:, k : k + 1])
                    ex_t = sbp.tile([128, K], F32, tag="ex")
                    nc.scalar.activation(out=ex_t[:], in_=e_t[:], func=AF.Exp)

                    ps_a = pst.tile([128, D + 1], F32, space="PSUM", tag="agg")
                    for k in range(K):
                        col = t * K + k
                        Oc = ohp.tile([128, 128], F32, tag="O")
                        nc.vector.tensor_scalar(
                            out=Oc[:], in0=iota_r[:],
                            scalar1=dstl_sb[:, col : col + 1],
                            scalar2=ex_t[:, k : k + 1],
                            op0=OP.is_equal, op1=OP.mult)
                        nc.tensor.matmul(
                            out=ps_a[:], lhsT=Oc[:], rhs=XL[:, k, :],
                            start=(k == 0), stop=(k == K - 1))

                    rec = sbp.tile([128, 1], F32, tag="rec")
                    nc.vector.reciprocal(rec[:], ps_a[:, D : D + 1])
                    h_sb = sbp.tile([128, D], F32, tag="h")
                    nc.vector.tensor_scalar(
                        out=h_sb[:], in0=ps_a[:, :D], scalar1=rec[:],
                        scalar2=None, op0=OP.mult)
                    if l < T - 1:
                        ps_t = pst.tile([128, 128], F32, space="PSUM", tag="tr")
                        nc.tensor.transpose(out=ps_t[:], in_=h_sb[:],
                                            identity=ident[:])
                        nc.scalar.activation(
                            out=hT[:, t * 128 : (t + 1) * 128], in_=ps_t[:],
                            func=AF.Relu, bias=bo_col[:], scale=1.0)
                    else:
                        o_sb = sbp.tile([128, D], F32, tag="o")
                        nc.vector.tensor_tensor(
                            out=o_sb[:], in0=h_sb[:], in1=bo_b[:], op=OP.add)
                        nc.sync.dma_start(
                            out=out_t[t * 128 : (t + 1) * 128, :], in_=o_sb[:])

    nc.compile()
    return nc


def _prep(inputs):
    x = np.asarray(inputs["x"], np.float32)
    ei = np.asarray(inputs["edge_index"]).astype(np.int64)
    Wl0 = np.asarray(inputs["Wl0"], np.float32)
    Wr0 = np.asarray(inputs["Wr0"], np.float32)
    bl0 = np.asarray(inputs["bl0"], np.float32)
    br0 = np.asarray(inputs["br0"], np.float32)
    Wl = np.asarray(inputs["Wl"], np.float32)
    Wr = np.asarray(inputs["Wr"], np.float32)
    bl = np.asarray(inputs["bl"], np.float32)
    br = np.asarray(inputs["br"], np.float32)
    att = np.asarray(inputs["att"], np.float32)
    bias = np.asarray(inputs["bias"], np.float32)

    loop = np.arange(N, dtype=np.int64)
    src = np.concatenate([ei[0], loop])
    dst = np.concatenate([ei[1], loop])

    owner = dst // SH
    local = dst - owner * SH

    # global padded row of each src node
    gsrc = (src // SH) * SHP + (src % SH)

    per_core = []
    max_cnt = 0
    for c in range(CORES):
        sel = owner == c
        s_g = gsrc[sel]
        s_loc = local[sel]
        order = np.argsort(s_loc, kind="stable")
        s_g = s_g[order]
        s_loc = s_loc[order]
        tid = s_loc >> 7
        counts = np.bincount(tid, minlength=TILES).astype(np.int64)
        # fake self-edges for pad nodes (local 6250..6271 -> tile 48)
        counts[TILES - 1] += SHP - SH
        max_cnt = max(max_cnt, int(counts.max()))
        per_core.append((s_g, s_loc, tid, counts))

    K = int(np.ceil(max_cnt / 128))

    srcs, dstrs, dstls = [], [], []
    for c in range(CORES):
        s_g, s_loc, tid, counts = per_core[c]
        src_arr = np.zeros((128, TILES * K), np.int32)
        dstr_arr = np.zeros((128, TILES * K), np.int32)
        dstl_arr = np.full((128, TILES * K), 200.0, np.float32)
        bounds = np.concatenate([[0], np.cumsum(np.bincount(tid, minlength=TILES))])
        for t in range(TILES):
            seg = slice(bounds[t], bounds[t + 1])
            n_e = bounds[t + 1] - bounds[t]
            e_g = s_g[seg]
            e_loc = s_loc[seg] & 127
            e_dstrow = s_loc[seg]
            if t == TILES - 1:
                # pad-node fake self-edges keep denominators nonzero
                pads = np.arange(SH, SHP, dtype=np.int64)
                e_g = np.concatenate([e_g, np.zeros(SHP - SH, np.int64)])
                e_loc = np.concatenate([e_loc, pads & 127])
                e_dstrow = np.concatenate([e_dstrow, pads])
                n_e += SHP - SH
            slot = np.arange(n_e)
            p = slot & 127
            k = slot >> 7
            src_arr[p, t * K + k] = e_g
            dstr_arr[p, t * K + k] = e_dstrow
            dstl_arr[p, t * K + k] = e_loc
        srcs.append(src_arr)
        dstrs.append(dstr_arr)
        dstls.append(dstl_arr)

    # weight / bias packing (biases folded: xl is bias-free, xr carries
    # bl+br for the score, output carries bias+bl)
    Wlr0 = np.concatenate([Wl0, Wr0], axis=1)
    Wlr = np.concatenate([Wl, Wr], axis=2)
    brow2 = np.stack([bl0 + br0] + [bl[i] + br[i] for i in range(T - 1)])
    bout = np.stack([bias[0] + bl0] + [bias[i + 1] + bl[i] for i in range(T - 1)]).T.copy()

    xT_full = np.zeros((DIN, NPG), np.float32)
    for c in range(CORES):
        xT_full[:, c * SHP : c * SHP + SH] = x[c * SH : (c + 1) * SH].T

    common = dict(Wlr0=Wlr0, Wlr=Wlr, brow2=brow2, bout=bout, attw=att,
                  xT_full=xT_full)
    in_maps = []
    for c in range(CORES):
        xT_own = np.zeros((DIN, SHP), np.float32)
        xT_own[:, :SH] = x[c * SH : (c + 1) * SH].T
        in_maps.append(dict(common, xT_own=xT_own, src_i=srcs[c],
                            dstr_i=dstrs[c], dstl_f=dstls[c],
                            dstl_r=np.ascontiguousarray(
                                dstls[c].T).reshape(1, -1)))
    return K, in_maps


_CACHE = {}


def kernel(**inputs) -> np.ndarray:
    out, _ = _run(inputs)
    return out


def _run(inputs, **kw):
    K, in_maps = _prep(inputs)
    if K not in _CACHE:
        _CACHE[K] = _build_nc(K)
    nc = _CACHE[K]
    res = run_bass_kernel_spmd(nc, in_maps, core_ids=list(range(CORES)), **kw)
    out = np.concatenate([res.results[c]["out"][:SH] for c in range(CORES)], axis=0)
    return out.astype(np.float32), res



# revision 9
# speedup vs baseline: 2.7161x; 2.7161x over previous
"""GATv2 x5 (gnn_message_passing) on 8 Trainium2 NeuronCores.

Sharding: nodes partitioned across 8 cores by destination-node owner
(6250 nodes/core, padded to 6272 = 49 tiles of 128). Edges live with
their dst owner, grouped into 128-edge chunks per dst-tile, split by
src half (local row < 3200 vs >= 3200) so the two per-layer AllGathers
can overlap edge compute. Per layer: each core computes xl/xr (bf16)
for its own nodes, AllGathers xl in two halves, then per pair of
dst-tiles: batched dma_gather of xl[src] and xr[dst] rows, GATv2 score
(LeakyReLU + att dot), segment softmax via exp + one-hot scatter
matmuls with a denominator column, and normalization. All matmuls and
gathers in bf16; accumulation, scores and output in fp32.
"""
import sys
import numpy as np

sys.path.insert(0, "/opt/trn_rl_repo")

import concourse.bass as bass
import concourse.bacc as bacc
import concourse.mybir as mybir
import concourse.tile as tile
from concourse.bass_utils import run_bass_kernel_spmd
from concourse.masks import make_identity

F32 = mybir.dt.float32
BF16 = mybir.dt.bfloat16
I16 = mybir.dt.int16
AF = mybir.ActivationFunctionType
OP = mybir.AluOpType

N = 50000
DIN = 7
D = 128
T = 5
CORES = 8
SH = N // CORES            # 6250 nodes per core
TILES = 49
SHP = TILES * 128          # 6272 padded nodes per core
H1 = 3200                  # local rows [0, 3200) -> AllGather half 1
H2 = SHP - H1              # 3072 rows          -> half 2
NEG = 0.2


def _build_nc(params):
    """params = (K1s, K2s): per-dst-tile chunk counts for src-half 1/2."""
    K1s, K2s = params
    CH = sum(K1s) + sum(K2s)   # total chunks per core

    # supertiles: pairs of dst tiles processed together
    groups = [(t, t + 1) for t in range(0, TILES - 1, 2)] + [(TILES - 1,)]
    self_qn = [0]   # round-robin SWDGE queue assignment

    nc = bacc.Bacc("TRN2", target_bir_lowering=False, debug=False,
                   num_devices=CORES, num_swdge_queues=4)

    xT_own = nc.dram_tensor("xT_own", [DIN, SHP], BF16, kind="ExternalInput")
    Wlr0 = nc.dram_tensor("Wlr0", [DIN, 2 * D], BF16, kind="ExternalInput")
    Wlr = nc.dram_tensor("Wlr", [T - 1, D, 2 * D], BF16, kind="ExternalInput")
    br2 = nc.dram_tensor("br2", [T, D], F32, kind="ExternalInput")
    bout = nc.dram_tensor("bout", [D, T], F32, kind="ExternalInput")
    attw = nc.dram_tensor("attw", [T, D], BF16, kind="ExternalInput")
    iota_in = nc.dram_tensor("iota_in", [1, 128], BF16, kind="ExternalInput")
    ixl_i = nc.dram_tensor("ixl", [128, CH * 8], I16, kind="ExternalInput")
    ixr_i = nc.dram_tensor("ixr", [128, CH * 8], I16, kind="ExternalInput")
    dstl_i = nc.dram_tensor("dstl", [128, CH], F32, kind="ExternalInput")

    out_t = nc.dram_tensor("out", [SHP, D], F32, kind="ExternalOutput")

    with tile.TileContext(nc) as tc:
        with (
            tc.tile_pool(name="pers", bufs=1) as pers,
            tc.tile_pool(name="wl", bufs=2) as wl,
            tc.tile_pool(name="edge", bufs=2) as ep,
            tc.tile_pool(name="sb", bufs=3) as sbp,
            tc.tile_pool(name="pro", bufs=2, space="PSUM") as psp,
            tc.tile_pool(name="aggn", bufs=2, space="PSUM") as psan,
            tc.tile_pool(name="aggd", bufs=2, space="PSUM") as psad,
            tc.tile_pool(name="tr", bufs=2, space="PSUM") as pst,
            tc.tile_pool(name="dram", bufs=2, space="DRAM") as dp,
        ):
            # --- persistent setup ---
            iota_b = pers.tile([128, 128], BF16)
            nc.sync.dma_start(out=iota_b[:],
                              in_=iota_in[0:1, :].partition_broadcast(128))
            ident = pers.tile([128, 128], BF16)
            make_identity(nc, ident[:])
            ixl_sb = pers.tile([128, CH * 8], I16)
            nc.sync.dma_start(out=ixl_sb[:], in_=ixl_i[:, :])
            ixr_sb = pers.tile([128, CH * 8], I16)
            nc.sync.dma_start(out=ixr_sb[:], in_=ixr_i[:, :])
            dstl_sb = pers.tile([128, CH], F32)
            nc.sync.dma_start(out=dstl_sb[:], in_=dstl_i[:, :])
            xT_sb = pers.tile([DIN, SHP], BF16)
            nc.sync.dma_start(out=xT_sb[:], in_=xT_own[:, :])
            ones_c = pers.tile([128, 1], BF16)
            nc.vector.memset(ones_c[:], 1.0)
            hT = pers.tile([128, SHP], BF16)

            for l in range(T):
                # --- per-layer constants ---
                w_sb = wl.tile([128, 2 * D], BF16, tag="w")
                if l == 0:
                    nc.sync.dma_start(out=w_sb[:DIN, :], in_=Wlr0[:, :])
                else:
                    nc.sync.dma_start(out=w_sb[:], in_=Wlr[l - 1, :, :])
                a_b = wl.tile([128, D], BF16, tag="ab")
                nc.sync.dma_start(
                    out=a_b[:], in_=attw[l : l + 1, :].partition_broadcast(128))
                br2_b = wl.tile([128, D], F32, tag="br2")
                nc.sync.dma_start(
                    out=br2_b[:], in_=br2[l : l + 1, :].partition_broadcast(128))
                bo_col = wl.tile([128, 1], F32, tag="boc")
                nc.sync.dma_start(out=bo_col[:], in_=bout[:, l : l + 1])
                if l == T - 1:
                    bo_b = wl.tile([128, D], F32, tag="bob")
                    nc.sync.dma_start(
                        out=bo_b[:],
                        in_=bout[:, l : l + 1].transpose([1, 0])
                        .partition_broadcast(128))

                # --- prologue: xl / xr for own nodes ---
                xl_cc = dp.tile([SHP, D], BF16, tag="xlcc")
                xr_dr = dp.tile([SHP, D], BF16, tag="xrdr")
                for m in range(TILES):
                    ps2 = psp.tile([128, 2 * D], F32, space="PSUM", tag="pro")
                    if l == 0:
                        lhsT = xT_sb[:, m * 128 : (m + 1) * 128]
                        rhs = w_sb[:DIN, :]
                    else:
                        lhsT = hT[:, m * 128 : (m + 1) * 128]
                        rhs = w_sb[:, :]
                    nc.tensor.matmul(out=ps2[:], lhsT=lhsT, rhs=rhs,
                                     start=True, stop=True)
                    xl_sb = sbp.tile([128, D], BF16, tag="xls")
                    nc.scalar.activation(out=xl_sb[:], in_=ps2[:, :D],
                                         func=AF.Identity)
                    nc.sync.dma_start(
                        out=xl_cc[m * 128 : (m + 1) * 128, :], in_=xl_sb[:])
                    xr_sb = sbp.tile([128, D], BF16, tag="xrs")
                    nc.vector.tensor_tensor(
                        out=xr_sb[:], in0=ps2[:, D:], in1=br2_b[:], op=OP.add)
                    nc.sync.dma_start(
                        out=xr_dr[m * 128 : (m + 1) * 128, :], in_=xr_sb[:])

                # --- AllGather xl in two halves ---
                xl_h1 = dp.tile([CORES * H1, D], BF16, tag="xlh1")
                nc.gpsimd.collective_compute(
                    "AllGather", OP.bypass,
                    replica_groups=[list(range(CORES))],
                    ins=[xl_cc[0:H1, :].opt()],
                    outs=[xl_h1[:, :].opt()],
                )
                xl_h2 = dp.tile([CORES * H2, D], BF16, tag="xlh2")
                nc.gpsimd.collective_compute(
                    "AllGather", OP.bypass,
                    replica_groups=[list(range(CORES))],
                    ins=[xl_cc[H1:SHP, :].opt()],
                    outs=[xl_h2[:, :].opt()],
                )

                # --- edge stage: per supertile (pair of dst tiles) ---
                pos = 0
                for ts in groups:
                    k1s = [K1s[t] for t in ts]
                    k2s = [K2s[t] for t in ts]
                    nA = sum(k1s)
                    nB = sum(k2s)
                    n = nA + nB
                    # chunk position -> (member index, dst-tile)
                    # layout: [A of ts[0] | A of ts[1] | B of ts[0] | B of ts[1]]
                    owner = []
                    for i, t in enumerate(ts):
                        owner += [i] * k1s[i]
                    for i, t in enumerate(ts):
                        owner += [i] * k2s[i]
                    first_chunk = {}
                    last_chunk = {}
                    for c, o in enumerate(owner):
                        if o not in first_chunk:
                            first_chunk[o] = c
                        last_chunk[o] = c

                    # dma_gather is capped at 1024 idxs (8 chunks of 128) by
                    # the SWDGE descriptor ring; split into sub-gathers spread
                    # over the 4 SWDGE queues.
                    def gathers(dst, src_ap, idx_sb, c0, nch):
                        off = 0
                        while off < nch:
                            g = min(8, nch - off)
                            nc.gpsimd.dma_gather(
                                dst[:, off : off + g, :], src_ap,
                                idx_sb[:, (c0 + off) * 8 : (c0 + off + g) * 8],
                                g * 128, g * 128, D,
                                queue_num=self_qn[0] % 4)
                            self_qn[0] += 1
                            off += g

                    XL = ep.tile([128, n, D], BF16, tag="XL")
                    if nA:
                        gathers(XL[:, :, :], xl_h1[:, :], ixl_sb, pos, nA)
                    if nB:
                        gathers(XL[:, nA:, :], xl_h2[:, :], ixl_sb,
                                pos + nA, nB)
                    XR = ep.tile([128, n, D], BF16, tag="XR")
                    gathers(XR[:, :, :], xr_dr[:, :], ixr_sb, pos, n)

                    S = ep.tile([128, n, D], BF16, tag="S")
                    L = ep.tile([128, n, D], BF16, tag="L")
                    e_t = sbp.tile([128, n], F32, tag="e")
                    ex_t = sbp.tile([128, n], F32, tag="ex")
                    scr = sbp.tile([128, D], BF16, tag="scr")
                    for lo, hi in ((0, nA), (nA, n)):
                        if lo == hi:
                            continue
                        nc.vector.tensor_tensor(
                            out=S[:, lo:hi, :], in0=XL[:, lo:hi, :],
                            in1=XR[:, lo:hi, :], op=OP.add)
                        nc.scalar.activation(
                            out=L[:, lo:hi, :], in_=S[:, lo:hi, :],
                            func=AF.Prelu, alpha=NEG)
                        for k in range(lo, hi):
                            nc.vector.scalar_tensor_tensor(
                                out=scr[:], in0=L[:, k, :], scalar=1.0,
                                in1=a_b[:], op0=OP.mult, op1=OP.mult,
                                accum_out=e_t[:, k : k + 1])
                        nc.scalar.activation(
                            out=ex_t[:, lo:hi], in_=e_t[:, lo:hi], func=AF.Exp)

                    Oc = ep.tile([128, n, 128], BF16, tag="Oc")
                    for k in range(n):
                        nc.vector.tensor_scalar(
                            out=Oc[:, k, :], in0=iota_b[:],
                            scalar1=dstl_sb[:, pos + k : pos + k + 1],
                            scalar2=ex_t[:, k : k + 1],
                            op0=OP.is_equal, op1=OP.mult)

                    ps_n = []
                    ps_d = []
                    for i in range(len(ts)):
                        pn = psan.tile([128, D], F32, space="PSUM",
                                       tag="aggn", name=f"psn{i}")
                        pd = psad.tile([128, 1], F32, space="PSUM",
                                       tag="aggd", name=f"psd{i}")
                        ps_n.append(pn)
                        ps_d.append(pd)
                    for c, o in enumerate(owner):
                        st = c == first_chunk[o]
                        sp = c == last_chunk[o]
                        nc.tensor.matmul(
                            out=ps_n[o][:, :], lhsT=Oc[:, c, :],
                            rhs=XL[:, c, :], start=st, stop=sp)
                        nc.tensor.matmul(
                            out=ps_d[o][:, :], lhsT=Oc[:, c, :],
                            rhs=ones_c[:], start=st, stop=sp)

                    for i, t in enumerate(ts):
                        rec = sbp.tile([128, 1], F32, tag="rec")
                        nc.vector.reciprocal(rec[:], ps_d[i][:, :])
                        if l < T - 1:
                            h_sb = sbp.tile([128, D], BF16, tag="h")
                            nc.vector.tensor_scalar(
                                out=h_sb[:], in0=ps_n[i][:, :], scalar1=rec[:],
                                scalar2=None, op0=OP.mult)
                            ps_tr = pst.tile([128, 128], BF16, space="PSUM",
                                             tag="tr")
                            nc.tensor.transpose(out=ps_tr[:], in_=h_sb[:],
                                                identity=ident[:])
                            nc.scalar.activation(
                                out=hT[:, t * 128 : (t + 1) * 128],
                                in_=ps_tr[:], func=AF.Relu, bias=bo_col[:],
                                scale=1.0)
                        else:
                            h_f = sbp.tile([128, D], F32, tag="hf")
                            nc.vector.tensor_scalar(
                                out=h_f[:], in0=ps_n[i][:, :], scalar1=rec[:],
                                scalar2=None, op0=OP.mult)
                            o_sb = sbp.tile([128, D], F32, tag="o")
                            nc.vector.tensor_tensor(
                                out=o_sb[:], in0=h_f[:], in1=bo_b[:], op=OP.add)
                            nc.sync.dma_start(
                                out=out_t[t * 128 : (t + 1) * 128, :],
                                in_=o_sb[:])
                    pos += n

    nc.compile()
    return nc


def _wrap_idx(idx_flat):
    """int16 idx vector -> [128, len/16] wrapped (16-partition) layout."""
    n = idx_flat.shape[0]
    assert n % 16 == 0
    w = idx_flat.reshape(n // 16, 16).T            # [16, n/16]
    return np.tile(w, (8, 1)).astype(np.int16)     # [128, n/16]


def _prep(inputs):
    x = np.asarray(inputs["x"], np.float32)
    ei = np.asarray(inputs["edge_index"]).astype(np.int64)
    Wl0 = np.asarray(inputs["Wl0"], np.float32)
    Wr0 = np.asarray(inputs["Wr0"], np.float32)
    bl0 = np.asarray(inputs["bl0"], np.float32)
    br0 = np.asarray(inputs["br0"], np.float32)
    Wl = np.asarray(inputs["Wl"], np.float32)
    Wr = np.asarray(inputs["Wr"], np.float32)
    bl = np.asarray(inputs["bl"], np.float32)
    br = np.asarray(inputs["br"], np.float32)
    att = np.asarray(inputs["att"], np.float32)
    bias = np.asarray(inputs["bias"], np.float32)

    loop = np.arange(N, dtype=np.int64)
    src = np.concatenate([ei[0], loop])
    dst = np.concatenate([ei[1], loop])

    owner = dst // SH
    local = dst - owner * SH          # 0..6249
    s_core = src // SH
    s_loc = src - s_core * SH         # 0..6249
    # gather row in half-1/2 tensors
    s_half = (s_loc >= H1).astype(np.int64)
    g1 = s_core * H1 + s_loc                   # valid when s_loc < H1
    g2 = s_core * H2 + (s_loc - H1)            # valid when s_loc >= H1

    # per (core, tile, half) edge counts to fix chunk structure
    tile_of = local >> 7
    cnt = np.zeros((CORES, TILES, 2), np.int64)
    np.add.at(cnt, (owner, tile_of, s_half), 1)
    # pad fake self-edges: local rows 6250..6271 on every core, tile 48,
    # src half 2 (padded rows >= H1)
    cnt[:, TILES - 1, 1] += SHP - SH

    K1s = tuple(int(v) for v in
                np.ceil(cnt[:, :, 0].max(axis=0) / 128).astype(np.int64))
    K2s = tuple(int(v) for v in
                np.ceil(cnt[:, :, 1].max(axis=0) / 128).astype(np.int64))

    groups = [(t, t + 1) for t in range(0, TILES - 1, 2)] + [(TILES - 1,)]
    CH = sum(K1s) + sum(K2s)

    # per-core packing
    ixls, ixrs, dstls = [], [], []
    for c in range(CORES):
        sel = owner == c
        e_tile = tile_of[sel]
        e_half = s_half[sel]
        e_g = np.where(e_half == 0, g1[sel], g2[sel])
        e_dloc = local[sel]                     # 0..6249
        # append pad fake self-edges
        pads = np.arange(SH, SHP, dtype=np.int64)
        e_tile = np.concatenate([e_tile, np.full(SHP - SH, TILES - 1)])
        e_half = np.concatenate([e_half, np.ones(SHP - SH, np.int64)])
        e_g = np.concatenate([e_g, c * H2 + (pads - H1)])
        e_dloc = np.concatenate([e_dloc, pads])

        # bucket by (tile, half)
        order = np.lexsort((e_dloc, e_half, e_tile))
        e_tile, e_half, e_g, e_dloc = (
            e_tile[order], e_half[order], e_g[order], e_dloc[order])
        bounds = np.searchsorted(
            e_tile * 2 + e_half, np.arange(TILES * 2 + 1))

        ixl = np.zeros(CH * 128, np.int64)
        ixr = np.zeros(CH * 128, np.int64)
        dstl = np.full((128, CH), 200.0, np.float32)
        pos = 0
        for ts in groups:
            for half, Ks in ((0, K1s), (1, K2s)):
                for t in ts:
                    kk = Ks[t]
                    b0, b1 = bounds[t * 2 + half], bounds[t * 2 + half + 1]
                    ne = b1 - b0
                    assert ne <= kk * 128
                    sl = slice(pos * 128, pos * 128 + ne)
                    ixl[sl] = e_g[b0:b1]
                    ixr[sl] = e_dloc[b0:b1]
                    i_in = np.arange(ne)
                    dstl[i_in & 127, pos + (i_in >> 7)] = e_dloc[b0:b1] & 127
                    pos += kk
        assert pos == CH
        assert ixl.max() < 32768 and ixr.max() < 32768
        ixls.append(_wrap_idx(ixl.astype(np.int16)))
        ixrs.append(_wrap_idx(ixr.astype(np.int16)))
        dstls.append(dstl)

    # weight / bias packing (biases folded: xl is bias-free, xr carries
    # bl+br for the score, output carries bias+bl)
    def bf16(a):
        import jax.numpy as jnp
        return np.asarray(jnp.asarray(np.asarray(a, np.float32),
                                      dtype=jnp.bfloat16))

    Wlr0 = bf16(np.concatenate([Wl0, Wr0], axis=1))
    Wlr_ = bf16(np.concatenate([Wl, Wr], axis=2))
    brow2 = np.stack([bl0 + br0] + [bl[i] + br[i] for i in range(T - 1)])
    bout = np.stack([bias[0] + bl0]
                    + [bias[i + 1] + bl[i] for i in range(T - 1)]).T.copy()
    iota_in = bf16(np.arange(128, dtype=np.float32)[None, :])

    common = dict(Wlr0=Wlr0, Wlr=Wlr_, br2=brow2.astype(np.float32),
                  bout=bout.astype(np.float32), attw=bf16(att),
                  iota_in=iota_in)
    in_maps = []
    for c in range(CORES):
        xT_own = np.zeros((DIN, SHP), np.float32)
        xT_own[:, :SH] = x[c * SH : (c + 1) * SH].T
        in_maps.append(dict(common, xT_own=bf16(xT_own), ixl=ixls[c],
                            ixr=ixrs[c], dstl=dstls[c]))
    return (K1s, K2s), in_maps


_CACHE = {}


def kernel(**inputs) -> np.ndarray:
    out, _ = _run(inputs)
    return out


def _run(inputs, **kw):
    params, in_maps = _prep(inputs)
    if params not in _CACHE:
        _CACHE[params] = _build_nc(params)
    nc = _CACHE[params]
    res = run_bass_kernel_spmd(nc, in_maps, core_ids=list(range(CORES)), **kw)
    out = np.concatenate([res.results[c]["out"][:SH] for c in range(CORES)],
                         axis=0)
    return out.astype(np.float32), res


# revision 14
# speedup vs baseline: 3.1625x; 1.1644x over previous
"""GATv2 x5 (gnn_message_passing) on 8 Trainium2 NeuronCores.

Sharding: nodes partitioned across 8 cores by destination-node owner
(6250 nodes/core, padded to 6272 = 49 tiles of 128). Edges live with
their dst owner, grouped into 128-edge chunks per dst-tile, split by
src half (local row < 3200 vs >= 3200) so the two per-layer AllGathers
can overlap edge compute. Per layer: each core computes xl/xr (bf16)
for its own nodes, AllGathers xl in two halves, then per pair of
dst-tiles: batched dma_gather of xl[src] and xr[dst] rows, GATv2 score
(LeakyReLU + att dot), segment softmax via exp + one-hot scatter
matmuls with a denominator column, and normalization. All matmuls and
gathers in bf16; accumulation, scores and output in fp32.
"""
import sys
import numpy as np

sys.path.insert(0, "/opt/trn_rl_repo")

import concourse.bass as bass
import concourse.bacc as bacc
import concourse.mybir as mybir
import concourse.tile as tile
from concourse.bass_utils import run_bass_kernel_spmd
from concourse.masks import make_identity

F32 = mybir.dt.float32
BF16 = mybir.dt.bfloat16
I16 = mybir.dt.int16
AF = mybir.ActivationFunctionType
OP = mybir.AluOpType

N = 50000
DIN = 7
D = 128
T = 5
CORES = 8
SH = N // CORES            # 6250 nodes per core
TILES = 49
SHP = TILES * 128          # 6272 padded nodes per core
NP_ALL = CORES * SHP       # 50176 slots globally
H1 = 4096                  # local rows [0, 4096) -> AllGather half 1
H2 = SHP - H1              # 2176 rows           -> half 2 (8*H1 = 32768
                           # so half-1 gather rows exactly fit int16)
NEG = 0.2


def _build_nc(params):
    """params = (K1s, K2s): per-dst-tile chunk counts for src-half 1/2."""
    K1s, K2s = params
    CH = sum(K1s) + sum(K2s)   # total chunks per core

    # supertiles: pairs of dst tiles processed together
    groups = [(t, t + 1) for t in range(0, TILES - 1, 2)] + [(TILES - 1,)]
    self_qn = [0]   # round-robin SWDGE queue assignment

    nc = bacc.Bacc("TRN2", target_bir_lowering=False, debug=False,
                   num_devices=CORES, num_swdge_queues=4)

    xT_own = nc.dram_tensor("xT_own", [DIN, SHP], BF16, kind="ExternalInput")
    Wlr0 = nc.dram_tensor("Wlr0", [DIN, 2 * D], BF16, kind="ExternalInput")
    Wlr = nc.dram_tensor("Wlr", [T - 1, D, 2 * D], BF16, kind="ExternalInput")
    br2 = nc.dram_tensor("br2", [T, D], F32, kind="ExternalInput")
    bout = nc.dram_tensor("bout", [D, T], F32, kind="ExternalInput")
    attw = nc.dram_tensor("attw", [T, D], BF16, kind="ExternalInput")
    iota_in = nc.dram_tensor("iota_in", [1, 128], BF16, kind="ExternalInput")
    ixl_i = nc.dram_tensor("ixl", [128, CH * 8], I16, kind="ExternalInput")
    ixr_i = nc.dram_tensor("ixr", [128, CH * 8], I16, kind="ExternalInput")
    dstl_i = nc.dram_tensor("dstl", [128, CH], F32, kind="ExternalInput")

    out_t = nc.dram_tensor("out", [SHP, D], F32, kind="ExternalOutput")

    with tile.TileContext(nc) as tc:
        with (
            tc.tile_pool(name="pers", bufs=1) as pers,
            tc.tile_pool(name="wl", bufs=2) as wl,
            tc.tile_pool(name="edge", bufs=2) as ep,
            tc.tile_pool(name="sb", bufs=3) as sbp,
            tc.tile_pool(name="pro", bufs=2, space="PSUM") as psp,
            tc.tile_pool(name="aggn", bufs=2, space="PSUM") as psan,
            tc.tile_pool(name="aggd", bufs=2, space="PSUM") as psad,
            tc.tile_pool(name="tr", bufs=2, space="PSUM") as pst,
            tc.tile_pool(name="dram", bufs=2, space="DRAM") as dp,
        ):
            # --- persistent setup ---
            iota_b = pers.tile([128, 128], BF16)
            nc.sync.dma_start(out=iota_b[:],
                              in_=iota_in[0:1, :].partition_broadcast(128))
            ident = pers.tile([128, 128], BF16)
            make_identity(nc, ident[:])
            ixl_sb = pers.tile([128, CH * 8], I16)
            nc.sync.dma_start(out=ixl_sb[:], in_=ixl_i[:, :])
            ixr_sb = pers.tile([128, CH * 8], I16)
            nc.sync.dma_start(out=ixr_sb[:], in_=ixr_i[:, :])
            dstl_sb = pers.tile([128, CH], F32)
            nc.sync.dma_start(out=dstl_sb[:], in_=dstl_i[:, :])
            xT_sb = pers.tile([DIN, SHP], BF16)
            nc.sync.dma_start(out=xT_sb[:], in_=xT_own[:, :])
            ones_c = pers.tile([128, 1], BF16)
            nc.vector.memset(ones_c[:], 1.0)
            hT = pers.tile([128, SHP], BF16)

            for l in range(T):
                # --- per-layer constants ---
                w_sb = wl.tile([128, 2 * D], BF16, tag="w")
                if l == 0:
                    nc.sync.dma_start(out=w_sb[:DIN, :], in_=Wlr0[:, :])
                else:
                    nc.sync.dma_start(out=w_sb[:], in_=Wlr[l - 1, :, :])
                a_b = wl.tile([128, D], BF16, tag="ab")
                nc.sync.dma_start(
                    out=a_b[:], in_=attw[l : l + 1, :].partition_broadcast(128))
                br2_b = wl.tile([128, D], F32, tag="br2")
                nc.sync.dma_start(
                    out=br2_b[:], in_=br2[l : l + 1, :].partition_broadcast(128))
                bo_col = wl.tile([128, 1], F32, tag="boc")
                nc.sync.dma_start(out=bo_col[:], in_=bout[:, l : l + 1])
                if l == T - 1:
                    bo_b = wl.tile([128, D], F32, tag="bob")
                    nc.sync.dma_start(
                        out=bo_b[:],
                        in_=bout[:, l : l + 1].transpose([1, 0])
                        .partition_broadcast(128))

                # --- prologue: xl / xr for own nodes ---
                xl_cc = dp.tile([SHP, D], BF16, tag="xlcc")
                xr_dr = dp.tile([SHP, D], BF16, tag="xrdr")
                for m in range(TILES):
                    ps2 = psp.tile([128, 2 * D], F32, space="PSUM", tag="pro")
                    if l == 0:
                        lhsT = xT_sb[:, m * 128 : (m + 1) * 128]
                        rhs = w_sb[:DIN, :]
                    else:
                        lhsT = hT[:, m * 128 : (m + 1) * 128]
                        rhs = w_sb[:, :]
                    nc.tensor.matmul(out=ps2[:], lhsT=lhsT, rhs=rhs,
                                     start=True, stop=True)
                    xl_sb = sbp.tile([128, D], BF16, tag="xls")
                    nc.scalar.activation(out=xl_sb[:], in_=ps2[:, :D],
                                         func=AF.Identity)
                    nc.sync.dma_start(
                        out=xl_cc[m * 128 : (m + 1) * 128, :], in_=xl_sb[:])
                    xr_sb = sbp.tile([128, D], BF16, tag="xrs")
                    nc.vector.tensor_tensor(
                        out=xr_sb[:], in0=ps2[:, D:], in1=br2_b[:], op=OP.add)
                    nc.sync.dma_start(
                        out=xr_dr[m * 128 : (m + 1) * 128, :], in_=xr_sb[:])

                # --- AllGather xl in two halves ---
                xl_h1 = dp.tile([CORES * H1, D], BF16, tag="xlh1")
                nc.gpsimd.collective_compute(
                    "AllGather", OP.bypass,
                    replica_groups=[list(range(CORES))],
                    ins=[xl_cc[0:H1, :].opt()],
                    outs=[xl_h1[:, :].opt()],
                )
                xl_h2 = dp.tile([CORES * H2, D], BF16, tag="xlh2")
                nc.gpsimd.collective_compute(
                    "AllGather", OP.bypass,
                    replica_groups=[list(range(CORES))],
                    ins=[xl_cc[H1:SHP, :].opt()],
                    outs=[xl_h2[:, :].opt()],
                )

                # --- edge stage: per supertile (pair of dst tiles) ---
                pos = 0
                for ts in groups:
                    k1s = [K1s[t] for t in ts]
                    k2s = [K2s[t] for t in ts]
                    nA = sum(k1s)
                    nB = sum(k2s)
                    n = nA + nB
                    # chunk position -> (member index, dst-tile)
                    # layout: [A of ts[0] | A of ts[1] | B of ts[0] | B of ts[1]]
                    owner = []
                    for i, t in enumerate(ts):
                        owner += [i] * k1s[i]
                    for i, t in enumerate(ts):
                        owner += [i] * k2s[i]
                    first_chunk = {}
                    last_chunk = {}
                    for c, o in enumerate(owner):
                        if o not in first_chunk:
                            first_chunk[o] = c
                        last_chunk[o] = c

                    # dma_gather is capped at 1024 idxs (8 chunks of 128) by
                    # the SWDGE descriptor ring; split into sub-gathers spread
                    # over the 4 SWDGE queues.
                    def gathers(dst, src_ap, idx_sb, c0, nch):
                        off = 0
                        while off < nch:
                            g = min(8, nch - off)
                            nc.gpsimd.dma_gather(
                                dst[:, off : off + g, :], src_ap,
                                idx_sb[:, (c0 + off) * 8 : (c0 + off + g) * 8],
                                g * 128, g * 128, D,
                                queue_num=self_qn[0] % 4)
                            self_qn[0] += 1
                            off += g

                    XL = ep.tile([128, n, D], BF16, tag="XL")
                    if nA:
                        gathers(XL[:, :, :], xl_h1[:, :], ixl_sb, pos, nA)
                    if nB:
                        gathers(XL[:, nA:, :], xl_h2[:, :], ixl_sb,
                                pos + nA, nB)
                    XR = ep.tile([128, n, D], BF16, tag="XR")
                    gathers(XR[:, :, :], xr_dr[:, :], ixr_sb, pos, n)

                    S = ep.tile([128, n, D], BF16, tag="S")
                    L = ep.tile([128, n, D], BF16, tag="L")
                    e_t = sbp.tile([128, n], F32, tag="e")
                    ex_t = sbp.tile([128, n], F32, tag="ex")
                    scr = sbp.tile([128, D], BF16, tag="scr")
                    for lo, hi in ((0, nA), (nA, n)):
                        if lo == hi:
                            continue
                        nc.vector.tensor_tensor(
                            out=S[:, lo:hi, :], in0=XL[:, lo:hi, :],
                            in1=XR[:, lo:hi, :], op=OP.add)
                        nc.scalar.activation(
                            out=L[:, lo:hi, :], in_=S[:, lo:hi, :],
                            func=AF.Prelu, alpha=NEG)
                        for k in range(lo, hi):
                            nc.vector.scalar_tensor_tensor(
                                out=scr[:], in0=L[:, k, :], scalar=1.0,
                                in1=a_b[:], op0=OP.mult, op1=OP.mult,
                                accum_out=e_t[:, k : k + 1])
                        nc.scalar.activation(
                            out=ex_t[:, lo:hi], in_=e_t[:, lo:hi], func=AF.Exp)

                    Oc = ep.tile([128, n, 128], BF16, tag="Oc")
                    for k in range(n):
                        nc.vector.tensor_scalar(
                            out=Oc[:, k, :], in0=iota_b[:],
                            scalar1=dstl_sb[:, pos + k : pos + k + 1],
                            scalar2=ex_t[:, k : k + 1],
                            op0=OP.is_equal, op1=OP.mult)

                    ps_n = []
                    ps_d = []
                    for i in range(len(ts)):
                        pn = psan.tile([128, D], F32, space="PSUM",
                                       tag="aggn", name=f"psn{i}")
                        pd = psad.tile([128, 1], F32, space="PSUM",
                                       tag="aggd", name=f"psd{i}")
                        ps_n.append(pn)
                        ps_d.append(pd)
                    for c, o in enumerate(owner):
                        st = c == first_chunk[o]
                        sp = c == last_chunk[o]
                        nc.tensor.matmul(
                            out=ps_n[o][:, :], lhsT=Oc[:, c, :],
                            rhs=XL[:, c, :], start=st, stop=sp)
                        nc.tensor.matmul(
                            out=ps_d[o][:, :], lhsT=Oc[:, c, :],
                            rhs=ones_c[:], start=st, stop=sp)

                    for i, t in enumerate(ts):
                        rec = sbp.tile([128, 1], F32, tag="rec")
                        nc.vector.reciprocal(rec[:], ps_d[i][:, :])
                        if l < T - 1:
                            h_sb = sbp.tile([128, D], BF16, tag="h")
                            nc.scalar.activation(
                                out=h_sb[:], in_=ps_n[i][:, :],
                                func=AF.Identity, scale=rec[:])
                            ps_tr = pst.tile([128, 128], BF16, space="PSUM",
                                             tag="tr")
                            nc.tensor.transpose(out=ps_tr[:], in_=h_sb[:],
                                                identity=ident[:])
                            nc.scalar.activation(
                                out=hT[:, t * 128 : (t + 1) * 128],
                                in_=ps_tr[:], func=AF.Relu, bias=bo_col[:],
                                scale=1.0)
                        else:
                            o_sb = sbp.tile([128, D], F32, tag="o")
                            nc.vector.scalar_tensor_tensor(
                                out=o_sb[:], in0=ps_n[i][:, :], scalar=rec[:],
                                in1=bo_b[:], op0=OP.mult, op1=OP.add)
                            nc.sync.dma_start(
                                out=out_t[t * 128 : (t + 1) * 128, :],
                                in_=o_sb[:])
                    pos += n

    nc.compile()
    return nc


def _wrap_idx(idx_flat):
    """int16 idx vector -> [128, len/16] wrapped (16-partition) layout."""
    n = idx_flat.shape[0]
    assert n % 16 == 0
    w = idx_flat.reshape(n // 16, 16).T            # [16, n/16]
    return np.tile(w, (8, 1)).astype(np.int16)     # [128, n/16]


def _balance(deg):
    """Greedy bin-pack NP_ALL nodes into 392 buckets of exactly 128 slots,
    equalizing per-bucket edge counts. Returns (core_of, loc_of)."""
    import heapq

    nb = CORES * TILES
    order = np.argsort(-deg, kind="stable")
    heap = [(0, b) for b in range(nb)]
    heapq.heapify(heap)
    slots_used = np.zeros(nb, np.int32)
    core_of = np.empty(NP_ALL, np.int32)
    loc_of = np.empty(NP_ALL, np.int32)
    for n in order:
        e, b = heapq.heappop(heap)
        core_of[n] = b // TILES
        loc_of[n] = (b % TILES) * 128 + slots_used[b]
        slots_used[b] += 1
        if slots_used[b] < 128:
            heapq.heappush(heap, (e + int(deg[n]), b))
    return core_of, loc_of


def _prep(inputs):
    x = np.asarray(inputs["x"], np.float32)
    ei = np.asarray(inputs["edge_index"]).astype(np.int64)
    Wl0 = np.asarray(inputs["Wl0"], np.float32)
    Wr0 = np.asarray(inputs["Wr0"], np.float32)
    bl0 = np.asarray(inputs["bl0"], np.float32)
    br0 = np.asarray(inputs["br0"], np.float32)
    Wl = np.asarray(inputs["Wl"], np.float32)
    Wr = np.asarray(inputs["Wr"], np.float32)
    bl = np.asarray(inputs["bl"], np.float32)
    br = np.asarray(inputs["br"], np.float32)
    att = np.asarray(inputs["att"], np.float32)
    bias = np.asarray(inputs["bias"], np.float32)

    # nodes 0..N-1 real, N..NP_ALL-1 virtual pads (degree-1 self loops keep
    # their softmax denominators finite)
    loop = np.arange(NP_ALL, dtype=np.int64)
    src = np.concatenate([ei[0], loop])
    dst = np.concatenate([ei[1], loop])

    deg = np.bincount(dst, minlength=NP_ALL)
    core_of, loc_of = _balance(deg)

    owner = core_of[dst].astype(np.int64)
    local = loc_of[dst].astype(np.int64)       # 0..SHP-1
    sc = core_of[src].astype(np.int64)
    sl = loc_of[src].astype(np.int64)
    s_half = (sl >= H1).astype(np.int64)
    g_row = np.where(s_half == 0, sc * H1 + sl, sc * H2 + (sl - H1))

    tile_of = local >> 7
    cnt = np.zeros((CORES, TILES, 2), np.int64)
    np.add.at(cnt, (owner, tile_of, s_half), 1)

    K1s = tuple(int(v) for v in
                np.ceil(cnt[:, :, 0].max(axis=0) / 128).astype(np.int64))
    K2s = tuple(int(v) for v in
                np.ceil(cnt[:, :, 1].max(axis=0) / 128).astype(np.int64))

    groups = [(t, t + 1) for t in range(0, TILES - 1, 2)] + [(TILES - 1,)]
    CH = sum(K1s) + sum(K2s)

    # per-core packing
    ixls, ixrs, dstls = [], [], []
    for c in range(CORES):
        sel = owner == c
        e_tile = tile_of[sel]
        e_half = s_half[sel]
        e_g = g_row[sel]
        e_dloc = local[sel]

        order = np.lexsort((e_dloc, e_half, e_tile))
        e_tile, e_half, e_g, e_dloc = (
            e_tile[order], e_half[order], e_g[order], e_dloc[order])
        bounds = np.searchsorted(
            e_tile * 2 + e_half, np.arange(TILES * 2 + 1))

        ixl = np.zeros(CH * 128, np.int64)
        ixr = np.zeros(CH * 128, np.int64)
        dstl = np.full((128, CH), 200.0, np.float32)
        pos = 0
        for ts in groups:
            for half, Ks in ((0, K1s), (1, K2s)):
                for t in ts:
                    kk = Ks[t]
                    b0, b1 = bounds[t * 2 + half], bounds[t * 2 + half + 1]
                    ne = b1 - b0
                    assert ne <= kk * 128
                    sl2 = slice(pos * 128, pos * 128 + ne)
                    ixl[sl2] = e_g[b0:b1]
                    ixr[sl2] = e_dloc[b0:b1]
                    i_in = np.arange(ne)
                    dstl[i_in & 127, pos + (i_in >> 7)] = e_dloc[b0:b1] & 127
                    pos += kk
        assert pos == CH
        assert ixl.max() < 32768 and ixr.max() < 32768
        ixls.append(_wrap_idx(ixl.astype(np.int16)))
        ixrs.append(_wrap_idx(ixr.astype(np.int16)))
        dstls.append(dstl)

    # weight / bias packing (biases folded: xl is bias-free, xr carries
    # bl+br for the score, output carries bias+bl)
    def bf16(a):
        import jax.numpy as jnp
        return np.asarray(jnp.asarray(np.asarray(a, np.float32),
                                      dtype=jnp.bfloat16))

    Wlr0 = bf16(np.concatenate([Wl0, Wr0], axis=1))
    Wlr_ = bf16(np.concatenate([Wl, Wr], axis=2))
    brow2 = np.stack([bl0 + br0] + [bl[i] + br[i] for i in range(T - 1)])
    bout = np.stack([bias[0] + bl0]
                    + [bias[i + 1] + bl[i] for i in range(T - 1)]).T.copy()
    iota_in = bf16(np.arange(128, dtype=np.float32)[None, :])

    common = dict(Wlr0=Wlr0, Wlr=Wlr_, br2=brow2.astype(np.float32),
                  bout=bout.astype(np.float32), attw=bf16(att),
                  iota_in=iota_in)
    in_maps = []
    nodes = np.arange(N)
    for c in range(CORES):
        xT_own = np.zeros((DIN, SHP), np.float32)
        m = core_of[:N] == c
        xT_own[:, loc_of[:N][m]] = x[nodes[m]].T
        in_maps.append(dict(common, xT_own=bf16(xT_own), ixl=ixls[c],
                            ixr=ixrs[c], dstl=dstls[c]))
    return (K1s, K2s), in_maps, (core_of, loc_of)


_CACHE = {}


def kernel(**inputs) -> np.ndarray:
    out, _ = _run(inputs)
    return out


def _run(inputs, **kw):
    params, in_maps, (core_of, loc_of) = _prep(inputs)
    if params not in _CACHE:
        _CACHE[params] = _build_nc(params)
    nc = _CACHE[params]
    res = run_bass_kernel_spmd(nc, in_maps, core_ids=list(range(CORES)), **kw)
    outs = np.stack([res.results[c]["out"] for c in range(CORES)])
    out = outs[core_of[:N], loc_of[:N]]
    return out.astype(np.float32), res
